# revision 18
# baseline (speedup 1.0000x reference)
"""BinaryDiff kernel for Trainium2 (8 NeuronCores) — bf16 end-to-end.

Computes out = x @ base + coeff * (x @ (2*mask - 1)) by folding the two
matmuls into one:  out = x @ W,  W = base + coeff*(2*mask - 1).

Sharding (8 cores = 2 row-groups x 4 col-groups):
  - x rows (B*S = 8192) split in 2 -> each core gets an x^T shard
    [4096 K, 4096 rows], pre-arranged on host in slab-major layout AND
    pre-cast to bf16, so every slab DMA is contiguous per partition and
    PE consumes it directly (no on-device conversion).
  - base/mask cols (4096) split in 4 -> per-core shards [4096, 1024]
    (base bf16, mask int8)
  - each core computes out shard [4096, 1024] fp32; host concatenates.

On-device per core:
  - W = bf16(bf16(base) + (2c*mask - c)) built once into resident SBUF
    ([128,32,1024] bf16) via ACT affine (int8->f32, runtime coeff via
    scale/bias APs) + DVE add.
  - x^T bf16 slabs [128,32,128] DMA'd straight into matmul position;
    32 m-strips x 2 n-halves x 32 k-chunks of bf16 matmuls (moving dim
    512) accumulate in fp32 across 8 PSUM banks.
  - ACT copies PSUM->SBUF (fp32), gpsimd DMAs results out.

Raw bass with manual semaphores. Two hard rules learned on this stack:
  1. Engine datapath instructions may carry at most ONE sync wait, so
     every wait is a standalone wait_ge on the consuming engine.
  2. DMA completions across different HW queues are unordered, so a
     cumulative semaphore over many in-flight DMAs is racy. DMAs use
     per-lane semaphores with at most one outstanding DMA per lane
     (enforced by the consumer-side slot gating). Engine completions
     retire in order, so cumulative per-engine semaphores are sound.

Startup-path optimizations (timeline-sim showed a single ~30us PE gap
at rep start, everything after it gap-free at the PE roofline):
  - DMA issue order interleaves the first W pieces with the 4 chase
    slabs (PE's first matmul needs strips 0-3 AND piece 0; queueing all
    8 slabs ahead of the first base/mask DMA stalled the PE).
  - 12 dummy matmuls on garbage SBUF warm the PE pstate during the
    startup DMA wait (ps[7] is reset by group 7's start=True, so the
    garbage is never observable).

Note: fp8e4 DoubleRow (2 k-tiles/instr @ 0.5 cyc/row nominal) was fully
prototyped with 3-term error compensation (xhi@Whi + xlo@Whi + xhi@Wlo,
rel err 2.9e-3) but measured SLOWER than bf16 on this hardware: a DR
matmul's 256-row stationary load serializes with its moving stream
(~200ns/instr vs bf16's 214 at 512 moving rows), so the 1.5x instruction
count of the compensated scheme loses; uncompensated fp8 fails the 2e-2
gate (~1e-1 full-scale). See kernel_fp8c.py.
"""
import contextlib

import numpy as np
import ml_dtypes

import concourse.bass as bass
import concourse.mybir as mybir
from concourse.bass_utils import run_bass_kernel_spmd

f32 = mybir.dt.float32
bf16 = mybir.dt.bfloat16
i8 = mybir.dt.int8
Copy = mybir.ActivationFunctionType.Copy
Identity = mybir.ActivationFunctionType.Identity

P = 128
B, S, D_IN, D_OUT = 4, 2048, 4096, 4096
ROWS = B * S                  # 8192
R_SHARDS, C_SHARDS = 2, 4
M = ROWS // R_SHARDS          # 4096 rows per core
NC = D_OUT // C_SHARDS        # 1024 cols per core
K = D_IN                      # 4096 contraction
KT = K // P                   # 32 k-chunks
MS = M // P                   # 32 m-strips
NH = NC // 512                # 2 n-halves
NT = 512
N_PIECES = KT                 # 32 W build pieces (one full-width [128,1024] per k)
N_GROUPS = MS * NH            # 64 output groups
SLAB_BUFS = 8
KH = KT // 2                  # k-chunks per half-slab DMA
CHUNK_BUFS = 4
OUT_BUFS = 4
PSB = 8                       # psum banks in rotation
XT_LANES = 16                 # half-slab DMA sem lanes (2 per slab buffer;
                              # 16 outstanding 512KB DMAs keep more rings busy)
PIECE_LANES = 8               # W piece DMA sem lanes (> CHUNK_BUFS)
OD_LANES = 8                  # out DMA sem lanes (> OUT_BUFS)


def _build_program(reps=1, kmult=1, out_f32=False):
    """reps > 1 repeats the whole pipeline inside one NEFF (for timing:
    T(reps=a) - T(reps=b) isolates (a-b) kernel bodies from dispatch
    overhead). Functionally identical output (each rep overwrites out).
    kmult > 1 issues every matmul kmult times (PE-rate probe; output is
    kmult times too large — timing use only)."""
    out_dt = f32 if out_f32 else bf16
    nc = bass.Bass()
    # xT arrives in slab-major bf16: xT_host[s, p, ko, i] = x[s*128+i, ko*128+p]
    # so each slab DMA reads 128 partitions x 8KB fully contiguous.
    xT = nc.declare_dram_parameter("xT", [MS * P, KT * P], bf16, isOutput=False)
    base = nc.declare_dram_parameter("base", [K, NC], bf16, isOutput=False)
    mask = nc.declare_dram_parameter("mask", [K, NC], i8, isOutput=False)
    coeff = nc.declare_dram_parameter("coeff", [P, 1], f32, isOutput=False)
    out = nc.declare_dram_parameter("out", [M, NC], out_dt, isOutput=True)

    xT3 = xT.rearrange("(s p) (ko i) -> s p ko i", p=P, i=P)
    base3 = base.rearrange("(ko p) n -> p ko n", p=P)
    mask3 = mask.rearrange("(ko p) n -> p ko n", p=P)
    out3 = out.rearrange("(mo p) n -> p mo n", p=P)

    with contextlib.ExitStack() as ctx:
        s_cdma = ctx.enter_context(nc.semaphore("s_cdma"))
        s_c2 = ctx.enter_context(nc.semaphore("s_c2"))
        s_xt = [ctx.enter_context(nc.semaphore(f"s_xt{i}"))
                for i in range(XT_LANES)]
        s_b = [ctx.enter_context(nc.semaphore(f"s_b{i}"))
               for i in range(PIECE_LANES)]
        s_m = [ctx.enter_context(nc.semaphore(f"s_m{i}"))
               for i in range(PIECE_LANES)]
        s_od = [ctx.enter_context(nc.semaphore(f"s_od{i}"))
                for i in range(OD_LANES)]
        s_s = ctx.enter_context(nc.semaphore("s_s"))      # ACT s-op done (1/piece)
        s_w = ctx.enter_context(nc.semaphore("s_w"))      # DVE w-op done (1/piece)
        s_mm = ctx.enter_context(nc.semaphore("s_mm"))    # PE group done (1/group)
        s_oc = ctx.enter_context(nc.semaphore("s_oc"))    # ACT out-copy done (1/group)

        w_sb = ctx.enter_context(nc.sbuf_tensor("w_sb", [P, KT, NC], bf16))
        xt_sb = ctx.enter_context(
            nc.sbuf_tensor("xt_sb", [P, SLAB_BUFS, KT, P], bf16))
        b_sb = ctx.enter_context(nc.sbuf_tensor("b_sb", [P, CHUNK_BUFS, NC], bf16))
        m_sb = ctx.enter_context(nc.sbuf_tensor("m_sb", [P, CHUNK_BUFS, NC], i8))
        sa_sb = ctx.enter_context(
            nc.sbuf_tensor("sa_sb", [P, CHUNK_BUFS, NC], bf16))
        o_sb = ctx.enter_context(
            nc.sbuf_tensor("o_sb", [P, OUT_BUFS, NT], out_dt))
        c_sb = ctx.enter_context(nc.sbuf_tensor("c_sb", [P, 1], f32))
        c2_sb = ctx.enter_context(nc.sbuf_tensor("c2_sb", [P, 1], f32))
        cn_sb = ctx.enter_context(nc.sbuf_tensor("cn_sb", [P, 1], f32))
        ps = [
            ctx.enter_context(nc.psum_tensor(f"ps{i}", [P, NT], f32))
            for i in range(PSB)
        ]

        with nc.Block() as block:

            @block.sync
            def _(sync):
                sync.dma_start(c_sb[:], coeff[:]).then_inc(s_cdma, 16)
                for it in range(reps):
                    bW = it * N_PIECES          # s_s/s_w base
                    bX = it * MS                # slab count base
                    bG = it * N_GROUPS
                    if it > 0:
                        # serialize rep boundaries so per-body timing equals a
                        # single-shot run (also keeps w_sb write/read ordered)
                        sync.wait_ge(s_oc, it * N_GROUPS)
                    # first slabs of this rep; slot s%SLAB_BUFS previously
                    # held strip s-SLAB_BUFS, free once PE finished its
                    # NH groups (s_mm counts one per group, in order).
                    # Each slab is fetched as two half-slab DMAs on separate
                    # lanes so twice as many rings run concurrently.
                    # Startup order interleaves the first W pieces with the
                    # chase slabs: the PE's first matmul needs strips 0-3 AND
                    # piece 0, so queueing all 8 slabs (8MB) ahead of the
                    # first base/mask DMA stalls the PE ~30us at rep start.
                    def emit_slab(s):
                        if bX + s >= SLAB_BUFS:
                            sync.wait_ge(s_mm, NH * (bX + s - SLAB_BUFS + 1))
                        for hf in range(2):
                            sync.dma_start(
                                xt_sb[:, s % SLAB_BUFS,
                                      hf * KH:(hf + 1) * KH],
                                xT3[s][:, hf * KH:(hf + 1) * KH],
                            ).then_inc(s_xt[(2 * s + hf) % XT_LANES], 16)

                    def emit_piece(j):
                        if bW + j >= CHUNK_BUFS:
                            sync.wait_ge(s_w, bW + j - CHUNK_BUFS + 1)
                            sync.wait_ge(s_s, bW + j - CHUNK_BUFS + 1)
                        sync.dma_start(
                            b_sb[:, j % CHUNK_BUFS], base3[:, j],
                        ).then_inc(s_b[j % PIECE_LANES], 16)
                        sync.dma_start(
                            m_sb[:, j % CHUNK_BUFS], mask3[:, j],
                        ).then_inc(s_m[j % PIECE_LANES], 16)

                    # pieces 0-1 and the 4 chase slabs first (PE's first
                    # need), then the full piece stream (feeds the chase at
                    # ~1.2us cadence), then slabs 4-7 (needed only after the
                    # chase) and the gated tail.
                    for j in range(2):
                        emit_piece(j)
                    for s in range(4):
                        emit_slab(s)
                    for j in range(2, N_PIECES):
                        emit_piece(j)
                    for s in range(4, min(SLAB_BUFS, MS)):
                        emit_slab(s)
                    # remaining slabs (two half-DMAs each)
                    for s in range(SLAB_BUFS, MS):
                        emit_slab(s)

            @block.scalar
            def _(scalar):
                scalar.wait_ge(s_cdma, 16)
                scalar.activation(c2_sb[:], c_sb[:], Copy, scale=2.0)
                scalar.activation(cn_sb[:], c_sb[:], Copy, scale=-1.0) \
                    .then_inc(s_c2, 1)
                # scale/bias operands are fetched at dispatch; wait for our own
                # writes to drain before the first use
                scalar.wait_ge(s_c2, 1)
                for it in range(reps):
                    bW = it * N_PIECES
                    bG = it * N_GROUPS
                    bP = it * (N_PIECES // PIECE_LANES) * 16
                    for j in range(N_PIECES):
                        scalar.wait_ge(s_m[j % PIECE_LANES],
                                       bP + 16 * (j // PIECE_LANES + 1))
                        if bW + j >= CHUNK_BUFS:
                            scalar.wait_ge(s_w, bW + j - CHUNK_BUFS + 1)
                        scalar.activation(
                            sa_sb[:, j % CHUNK_BUFS], m_sb[:, j % CHUNK_BUFS],
                            Identity, scale=c2_sb[:], bias=cn_sb[:],
                        ).then_inc(s_s, 1)
                    # PSUM -> SBUF copies
                    for g in range(N_GROUPS):
                        scalar.wait_ge(s_mm, bG + g + 1)
                        if bG + g >= OUT_BUFS:
                            gp = bG + g - OUT_BUFS
                            scalar.wait_ge(s_od[gp % OD_LANES],
                                           16 * (gp // OD_LANES + 1))
                        scalar.copy(o_sb[:, g % OUT_BUFS], ps[g % PSB][:]) \
                            .then_inc(s_oc, 1)

            @block.vector
            def _(vector):
                for it in range(reps):
                    bW = it * N_PIECES
                    bP = it * (N_PIECES // PIECE_LANES) * 16
                    for j in range(N_PIECES):
                        vector.wait_ge(s_s, bW + j + 1)
                        vector.wait_ge(s_b[j % PIECE_LANES],
                                       bP + 16 * (j // PIECE_LANES + 1))
                        vector.tensor_tensor(
                            w_sb[:, j, :],
                            sa_sb[:, j % CHUNK_BUFS], b_sb[:, j % CHUNK_BUFS],
                            mybir.AluOpType.add,
                        ).then_inc(s_w, 1)

            @block.tensor
            def _(tensor):
                # pstate warmup: dummy matmuls on garbage SBUF during the
                # startup DMA gap so the first real matmuls run at max clock.
                # ps[7]'s first real use is group 7 with start=True, which
                # resets the bank, so the garbage results are never read.
                for d in range(12):
                    tensor.matmul(
                        ps[7][:], xt_sb[:, 7, d, :], w_sb[:, d, :NT],
                        start=True, stop=True, skip_group_check=True,
                    )
                for it in range(reps):
                    bW = it * N_PIECES
                    bX = it * MS
                    bG = it * N_GROUPS
                    bL = it * (2 * MS // XT_LANES) * 16
                    # strips 0-3 fused k-major across all 8 psum banks: 8 mms
                    # of PE work per W piece keeps PE busy while the
                    # W build streams in
                    NCH = PSB // NH   # chase strips
                    for st in range(NCH):
                        tensor.wait_ge(s_xt[(2 * st) % XT_LANES], bL + 16)
                        tensor.wait_ge(s_xt[(2 * st + 1) % XT_LANES],
                                       bL + 16)
                    for k in range(KT):
                        tensor.wait_ge(s_w, bW + k + 1)
                        for st in range(NCH):
                            for h in range(NH):
                                g = bG + NH * st + h
                                if k == 0 and g >= PSB:
                                    tensor.wait_ge(s_oc, g - PSB + 1)
                                for q in range(kmult):
                                    mm = tensor.matmul(
                                        ps[g % PSB][:], xt_sb[:, st, k, :],
                                        w_sb[:, k, h * NT:(h + 1) * NT],
                                        start=(k == 0 and q == 0),
                                        stop=(k == KT - 1 and q == kmult - 1),
                                    )
                                    if k == KT - 1 and q == kmult - 1:
                                        # stops fire in group order 0..7
                                        mm.then_inc(s_mm, 1)
                    for strip in range(NCH, MS):
                        tensor.wait_ge(s_xt[(2 * strip) % XT_LANES],
                                       bL + 16 * (strip // SLAB_BUFS + 1))
                        tensor.wait_ge(s_xt[(2 * strip + 1) % XT_LANES],
                                       bL + 16 * (strip // SLAB_BUFS + 1))
                        for h in range(NH):
                            g = bG + NH * strip + h
                            if g >= PSB:
                                tensor.wait_ge(s_oc, g - PSB + 1)
                            for k in range(KT):
                                for q in range(kmult):
                                    mm = tensor.matmul(
                                        ps[g % PSB][:],
                                        xt_sb[:, strip % SLAB_BUFS, k, :],
                                        w_sb[:, k, h * NT:(h + 1) * NT],
                                        start=(k == 0 and q == 0),
                                        stop=(k == KT - 1 and q == kmult - 1),
                                    )
                                    if k == KT - 1 and q == kmult - 1:
                                        mm.then_inc(s_mm, 1)

            @block.gpsimd
            def _(gpsimd):
                for it in range(reps):
                    bG = it * N_GROUPS
                    for g in range(N_GROUPS):
                        strip, h = g // NH, g % NH
                        gpsimd.wait_ge(s_oc, bG + g + 1)
                        gpsimd.dma_start(
                            out3[:, strip, h * NT:(h + 1) * NT],
                            o_sb[:, g % OUT_BUFS],
                        ).then_inc(s_od[g % OD_LANES], 16)
                for i in range(OD_LANES):
                    cnt = (reps * N_GROUPS - 1 - i) // OD_LANES + 1
                    gpsimd.wait_ge(s_od[i], 16 * cnt)

    return nc


def make_in_maps(x, base, coeff, mask):
    """Host-side shard + layout prep. x: [ROWS, K] f32 (already reshaped)."""
    in_maps = []
    shard_ids = []
    for r in range(R_SHARDS):
        x_r = x[r * M:(r + 1) * M, :]
        # slab-major: [s, p, ko, i] = x_r[s*128+i, ko*128+p], cast to bf16
        xT_r = np.ascontiguousarray(
            x_r.reshape(MS, P, KT, P).transpose(0, 3, 2, 1)
        ).reshape(MS * P, KT * P).astype(ml_dtypes.bfloat16)
        for c in range(C_SHARDS):
            in_maps.append({
                "xT": xT_r,
                "base": np.ascontiguousarray(
                    base[:, c * NC:(c + 1) * NC]).astype(ml_dtypes.bfloat16),
                "mask": np.ascontiguousarray(
                    mask[:, c * NC:(c + 1) * NC]).astype(np.int8),
                "coeff": np.full((P, 1), np.float32(coeff), dtype=np.float32),
            })
            shard_ids.append((r, c))
    return in_maps, shard_ids


_PROG = None


def kernel(x, base, coeff, mask):
    global _PROG
    if _PROG is None:
        _PROG = _build_program()

    x = np.asarray(x, dtype=np.float32).reshape(ROWS, K)
    base = np.asarray(base, dtype=np.float32)
    mask = np.asarray(mask, dtype=np.int32)

    in_maps, shard_ids = make_in_maps(x, base, coeff, mask)
    res = run_bass_kernel_spmd(_PROG, in_maps, list(range(8))).results

    out = np.empty((ROWS, D_OUT), dtype=np.float32)
    for i, (r, c) in enumerate(shard_ids):
        out[r * M:(r + 1) * M, c * NC:(c + 1) * NC] = \
            np.asarray(res[i]["out"]).astype(np.float32)
    return out.reshape(B, S, D_OUT)



# revision 20
# speedup vs baseline: 1.0338x; 1.0338x over previous
"""BinaryDiff kernel for Trainium2 (8 NeuronCores) — bf16 end-to-end.

Computes out = x @ base + coeff * (x @ (2*mask - 1)) by folding the two
matmuls into one:  out = x @ W,  W = base + coeff*(2*mask - 1).

Sharding (8 cores = 2 row-groups x 4 col-groups):
  - x rows (B*S = 8192) split in 2 -> each core gets an x^T shard
    [4096 K, 4096 rows], pre-arranged on host in slab-major layout AND
    pre-cast to bf16, so every slab DMA is contiguous per partition and
    PE consumes it directly (no on-device conversion).
  - base/mask cols (4096) split in 4 -> per-core shards [4096, 1024]
    (base bf16, mask int8)
  - each core computes out shard [4096, 1024] fp32; host concatenates.

On-device per core:
  - W = bf16(bf16(base) + (2c*mask - c)) built once into resident SBUF
    ([128,32,1024] bf16) via ACT affine (int8->f32, runtime coeff via
    scale/bias APs) + DVE add.
  - x^T bf16 slabs [128,32,128] DMA'd straight into matmul position;
    32 m-strips x 2 n-halves x 32 k-chunks of bf16 matmuls (moving dim
    512) accumulate in fp32 across 8 PSUM banks.
  - ACT copies PSUM->SBUF (fp32), gpsimd DMAs results out.

Raw bass with manual semaphores. Two hard rules learned on this stack:
  1. Engine datapath instructions may carry at most ONE sync wait, so
     every wait is a standalone wait_ge on the consuming engine.
  2. DMA completions across different HW queues are unordered, so a
     cumulative semaphore over many in-flight DMAs is racy. DMAs use
     per-lane semaphores with at most one outstanding DMA per lane
     (enforced by the consumer-side slot gating). Engine completions
     retire in order, so cumulative per-engine semaphores are sound.

Startup-path optimizations (timeline-sim showed a single ~30us PE gap
at rep start, everything after it gap-free at the PE roofline):
  - DMA issue order interleaves the first W pieces with the 4 chase
    slabs (PE's first matmul needs strips 0-3 AND piece 0; queueing all
    8 slabs ahead of the first base/mask DMA stalled the PE).
  - 12 dummy matmuls on garbage SBUF warm the PE pstate during the
    startup DMA wait (ps[7] is reset by group 7's start=True, so the
    garbage is never observable).

Note: fp8e4 DoubleRow (2 k-tiles/instr @ 0.5 cyc/row nominal) was fully
prototyped with 3-term error compensation (xhi@Whi + xlo@Whi + xhi@Wlo,
rel err 2.9e-3) but measured SLOWER than bf16 on this hardware: a DR
matmul's 256-row stationary load serializes with its moving stream
(~200ns/instr vs bf16's 214 at 512 moving rows), so the 1.5x instruction
count of the compensated scheme loses; uncompensated fp8 fails the 2e-2
gate (~1e-1 full-scale). See kernel_fp8c.py.
"""
import contextlib

import numpy as np
import ml_dtypes

import concourse.bass as bass
import concourse.mybir as mybir
from concourse.bass_utils import run_bass_kernel_spmd

f32 = mybir.dt.float32
bf16 = mybir.dt.bfloat16
i8 = mybir.dt.int8
Copy = mybir.ActivationFunctionType.Copy
Identity = mybir.ActivationFunctionType.Identity

P = 128
B, S, D_IN, D_OUT = 4, 2048, 4096, 4096
ROWS = B * S                  # 8192
R_SHARDS, C_SHARDS = 2, 4
M = ROWS // R_SHARDS          # 4096 rows per core
NC = D_OUT // C_SHARDS        # 1024 cols per core
K = D_IN                      # 4096 contraction
KT = K // P                   # 32 k-chunks
MS = M // P                   # 32 m-strips
NH = NC // 512                # 2 n-halves
NT = 512
N_PIECES = KT                 # 32 W build pieces (one full-width [128,1024] per k)
N_GROUPS = MS * NH            # 64 output groups
SLAB_BUFS = 8
KH = KT // 2                  # k-chunks per half-slab DMA
CHUNK_BUFS = 4
OUT_BUFS = 4
PSB = 8                       # psum banks in rotation
XT_LANES = 16                 # half-slab DMA sem lanes (2 per slab buffer;
                              # 16 outstanding 512KB DMAs keep more rings busy)
PIECE_LANES = 8               # W piece DMA sem lanes (> CHUNK_BUFS)
OD_LANES = 8                  # out DMA sem lanes (> OUT_BUFS)


def _build_program(reps=1, kmult=1, out_f32=False):
    """reps > 1 repeats the whole pipeline inside one NEFF (for timing:
    T(reps=a) - T(reps=b) isolates (a-b) kernel bodies from dispatch
    overhead). Functionally identical output (each rep overwrites out).
    kmult > 1 issues every matmul kmult times (PE-rate probe; output is
    kmult times too large — timing use only)."""
    out_dt = f32 if out_f32 else bf16
    nc = bass.Bass()
    # xT arrives in slab-major bf16: xT_host[s, p, ko, i] = x[s*128+i, ko*128+p]
    # so each slab DMA reads 128 partitions x 8KB fully contiguous.
    xT = nc.declare_dram_parameter("xT", [MS * P, KT * P], bf16, isOutput=False)
    base = nc.declare_dram_parameter("base", [K, NC], bf16, isOutput=False)
    mask = nc.declare_dram_parameter("mask", [K, NC], i8, isOutput=False)
    coeff = nc.declare_dram_parameter("coeff", [P, 1], f32, isOutput=False)
    out = nc.declare_dram_parameter("out", [M, NC], out_dt, isOutput=True)

    xT3 = xT.rearrange("(s p) (ko i) -> s p ko i", p=P, i=P)
    base3 = base.rearrange("(ko p) n -> p ko n", p=P)
    mask3 = mask.rearrange("(ko p) n -> p ko n", p=P)
    out3 = out.rearrange("(mo p) n -> p mo n", p=P)

    with contextlib.ExitStack() as ctx:
        s_cdma = ctx.enter_context(nc.semaphore("s_cdma"))
        s_c2 = ctx.enter_context(nc.semaphore("s_c2"))
        s_xt = [ctx.enter_context(nc.semaphore(f"s_xt{i}"))
                for i in range(XT_LANES)]
        s_b = [ctx.enter_context(nc.semaphore(f"s_b{i}"))
               for i in range(PIECE_LANES)]
        s_m = [ctx.enter_context(nc.semaphore(f"s_m{i}"))
               for i in range(PIECE_LANES)]
        s_od = [ctx.enter_context(nc.semaphore(f"s_od{i}"))
                for i in range(OD_LANES)]
        s_s = ctx.enter_context(nc.semaphore("s_s"))      # ACT s-op done (1/piece)
        s_w = ctx.enter_context(nc.semaphore("s_w"))      # DVE w-op done (1/piece)
        s_mm = ctx.enter_context(nc.semaphore("s_mm"))    # PE group done (1/group)
        s_oc = ctx.enter_context(nc.semaphore("s_oc"))    # ACT out-copy done (1/group)

        w_sb = ctx.enter_context(nc.sbuf_tensor("w_sb", [P, KT, NC], bf16))
        xt_sb = ctx.enter_context(
            nc.sbuf_tensor("xt_sb", [P, SLAB_BUFS, KT, P], bf16))
        b_sb = ctx.enter_context(nc.sbuf_tensor("b_sb", [P, CHUNK_BUFS, NC], bf16))
        m_sb = ctx.enter_context(nc.sbuf_tensor("m_sb", [P, CHUNK_BUFS, NC], i8))
        sa_sb = ctx.enter_context(
            nc.sbuf_tensor("sa_sb", [P, CHUNK_BUFS, NC], bf16))
        o_sb = ctx.enter_context(
            nc.sbuf_tensor("o_sb", [P, OUT_BUFS, NT], out_dt))
        c_sb = ctx.enter_context(nc.sbuf_tensor("c_sb", [P, 1], f32))
        c2_sb = ctx.enter_context(nc.sbuf_tensor("c2_sb", [P, 1], f32))
        cn_sb = ctx.enter_context(nc.sbuf_tensor("cn_sb", [P, 1], f32))
        ps = [
            ctx.enter_context(nc.psum_tensor(f"ps{i}", [P, NT], f32))
            for i in range(PSB)
        ]

        with nc.Block() as block:

            @block.sync
            def _(sync):
                sync.dma_start(c_sb[:], coeff[:]).then_inc(s_cdma, 16)
                for it in range(reps):
                    bW = it * N_PIECES          # s_s/s_w base
                    bX = it * MS                # slab count base
                    bG = it * N_GROUPS
                    if it > 0:
                        # serialize rep boundaries so per-body timing equals a
                        # single-shot run (also keeps w_sb write/read ordered)
                        sync.wait_ge(s_oc, it * N_GROUPS)
                    # first slabs of this rep; slot s%SLAB_BUFS previously
                    # held strip s-SLAB_BUFS, free once PE finished its
                    # NH groups (s_mm counts one per group, in order).
                    # Each slab is fetched as two half-slab DMAs on separate
                    # lanes so twice as many rings run concurrently.
                    # Startup order interleaves the first W pieces with the
                    # chase slabs: the PE's first matmul needs strips 0-3 AND
                    # piece 0, so queueing all 8 slabs (8MB) ahead of the
                    # first base/mask DMA stalls the PE ~30us at rep start.
                    def emit_slab(s):
                        if bX + s >= SLAB_BUFS:
                            sync.wait_ge(s_mm, NH * (bX + s - SLAB_BUFS + 1))
                        for hf in range(2):
                            sync.dma_start(
                                xt_sb[:, s % SLAB_BUFS,
                                      hf * KH:(hf + 1) * KH],
                                xT3[s][:, hf * KH:(hf + 1) * KH],
                            ).then_inc(s_xt[(2 * s + hf) % XT_LANES], 16)

                    def emit_piece(j):
                        if bW + j >= CHUNK_BUFS:
                            sync.wait_ge(s_w, bW + j - CHUNK_BUFS + 1)
                            sync.wait_ge(s_s, bW + j - CHUNK_BUFS + 1)
                        sync.dma_start(
                            b_sb[:, j % CHUNK_BUFS], base3[:, j],
                        ).then_inc(s_b[j % PIECE_LANES], 16)
                        sync.dma_start(
                            m_sb[:, j % CHUNK_BUFS], mask3[:, j],
                        ).then_inc(s_m[j % PIECE_LANES], 16)

                    # pieces 0-1 and the 4 chase slabs first (PE's first
                    # need), then the full piece stream (feeds the chase at
                    # ~1.2us cadence), then slabs 4-7 (needed only after the
                    # chase) and the gated tail.
                    for j in range(2):
                        emit_piece(j)
                    for s in range(4):
                        emit_slab(s)
                    for j in range(2, N_PIECES):
                        emit_piece(j)
                    for s in range(4, min(SLAB_BUFS, MS)):
                        emit_slab(s)
                    # remaining slabs (two half-DMAs each)
                    for s in range(SLAB_BUFS, MS):
                        emit_slab(s)

            @block.scalar
            def _(scalar):
                scalar.wait_ge(s_cdma, 16)
                scalar.activation(c2_sb[:], c_sb[:], Copy, scale=2.0)
                scalar.activation(cn_sb[:], c_sb[:], Copy, scale=-1.0) \
                    .then_inc(s_c2, 1)
                # scale/bias operands are fetched at dispatch; wait for our own
                # writes to drain before the first use
                scalar.wait_ge(s_c2, 1)
                for it in range(reps):
                    bW = it * N_PIECES
                    bG = it * N_GROUPS
                    bP = it * (N_PIECES // PIECE_LANES) * 16
                    for j in range(N_PIECES):
                        scalar.wait_ge(s_m[j % PIECE_LANES],
                                       bP + 16 * (j // PIECE_LANES + 1))
                        if bW + j >= CHUNK_BUFS:
                            scalar.wait_ge(s_w, bW + j - CHUNK_BUFS + 1)
                        scalar.activation(
                            sa_sb[:, j % CHUNK_BUFS], m_sb[:, j % CHUNK_BUFS],
                            Identity, scale=c2_sb[:], bias=cn_sb[:],
                        ).then_inc(s_s, 1)
                    # PSUM -> SBUF copies
                    for g in range(N_GROUPS):
                        scalar.wait_ge(s_mm, bG + g + 1)
                        if bG + g >= OUT_BUFS:
                            gp = bG + g - OUT_BUFS
                            scalar.wait_ge(s_od[gp % OD_LANES],
                                           16 * (gp // OD_LANES + 1))
                        scalar.copy(o_sb[:, g % OUT_BUFS], ps[g % PSB][:]) \
                            .then_inc(s_oc, 1)

            @block.vector
            def _(vector):
                for it in range(reps):
                    bW = it * N_PIECES
                    bP = it * (N_PIECES // PIECE_LANES) * 16
                    for j in range(N_PIECES):
                        vector.wait_ge(s_s, bW + j + 1)
                        vector.wait_ge(s_b[j % PIECE_LANES],
                                       bP + 16 * (j // PIECE_LANES + 1))
                        vector.tensor_tensor(
                            w_sb[:, j, :],
                            sa_sb[:, j % CHUNK_BUFS], b_sb[:, j % CHUNK_BUFS],
                            mybir.AluOpType.add,
                        ).then_inc(s_w, 1)

            @block.tensor
            def _(tensor):
                # pstate warmup: dummy matmuls on garbage SBUF during the
                # startup DMA gap so the first real matmuls run at max clock.
                # ps[7]'s first real use is group 7 with start=True, which
                # resets the bank, so the garbage results are never read.
                for d in range(12):
                    tensor.matmul(
                        ps[7][:], xt_sb[:, 7, d, :], w_sb[:, d, :NT],
                        start=True, stop=True, skip_group_check=True,
                    )
                for it in range(reps):
                    bW = it * N_PIECES
                    bX = it * MS
                    bG = it * N_GROUPS
                    bL = it * (2 * MS // XT_LANES) * 16
                    # strips 0-3 fused k-major across all 8 psum banks: 8 mms
                    # of PE work per W piece keeps PE busy while the
                    # W build streams in
                    NCH = PSB // NH   # chase strips
                    for st in range(NCH):
                        tensor.wait_ge(s_xt[(2 * st) % XT_LANES], bL + 16)
                        tensor.wait_ge(s_xt[(2 * st + 1) % XT_LANES],
                                       bL + 16)
                    for k in range(KT):
                        tensor.wait_ge(s_w, bW + k + 1)
                        for st in range(NCH):
                            for h in range(NH):
                                g = bG + NH * st + h
                                if k == 0 and g >= PSB:
                                    tensor.wait_ge(s_oc, g - PSB + 1)
                                for q in range(kmult):
                                    mm = tensor.matmul(
                                        ps[g % PSB][:], xt_sb[:, st, k, :],
                                        w_sb[:, k, h * NT:(h + 1) * NT],
                                        start=(k == 0 and q == 0),
                                        stop=(k == KT - 1 and q == kmult - 1),
                                    )
                                    if k == KT - 1 and q == kmult - 1:
                                        # stops fire in group order 0..7
                                        mm.then_inc(s_mm, 1)
                    for strip in range(NCH, MS):
                        tensor.wait_ge(s_xt[(2 * strip) % XT_LANES],
                                       bL + 16 * (strip // SLAB_BUFS + 1))
                        tensor.wait_ge(s_xt[(2 * strip + 1) % XT_LANES],
                                       bL + 16 * (strip // SLAB_BUFS + 1))
                        for h in range(NH):
                            g = bG + NH * strip + h
                            if g >= PSB:
                                tensor.wait_ge(s_oc, g - PSB + 1)
                            for k in range(KT):
                                for q in range(kmult):
                                    mm = tensor.matmul(
                                        ps[g % PSB][:],
                                        xt_sb[:, strip % SLAB_BUFS, k, :],
                                        w_sb[:, k, h * NT:(h + 1) * NT],
                                        start=(k == 0 and q == 0),
                                        stop=(k == KT - 1 and q == kmult - 1),
                                    )
                                    if k == KT - 1 and q == kmult - 1:
                                        mm.then_inc(s_mm, 1)

            @block.gpsimd
            def _(gpsimd):
                for it in range(reps):
                    bG = it * N_GROUPS
                    for g in range(N_GROUPS):
                        strip, h = g // NH, g % NH
                        gpsimd.wait_ge(s_oc, bG + g + 1)
                        gpsimd.dma_start(
                            out3[:, strip, h * NT:(h + 1) * NT],
                            o_sb[:, g % OUT_BUFS],
                        ).then_inc(s_od[g % OD_LANES], 16)
                for i in range(OD_LANES):
                    cnt = (reps * N_GROUPS - 1 - i) // OD_LANES + 1
                    gpsimd.wait_ge(s_od[i], 16 * cnt)

    return nc


def make_in_maps(x, base, coeff, mask):
    """Host-side shard + layout prep. x: [ROWS, K] f32 (already reshaped)."""
    in_maps = []
    shard_ids = []
    for r in range(R_SHARDS):
        x_r = x[r * M:(r + 1) * M, :]
        # slab-major: [s, p, ko, i] = x_r[s*128+i, ko*128+p], cast to bf16
        xT_r = np.ascontiguousarray(
            x_r.reshape(MS, P, KT, P).transpose(0, 3, 2, 1)
        ).reshape(MS * P, KT * P).astype(ml_dtypes.bfloat16)
        for c in range(C_SHARDS):
            in_maps.append({
                "xT": xT_r,
                "base": np.ascontiguousarray(
                    base[:, c * NC:(c + 1) * NC]).astype(ml_dtypes.bfloat16),
                "mask": np.ascontiguousarray(
                    mask[:, c * NC:(c + 1) * NC]).astype(np.int8),
                "coeff": np.full((P, 1), np.float32(coeff), dtype=np.float32),
            })
            shard_ids.append((r, c))
    return in_maps, shard_ids


_PROG = None


def kernel(x, base, coeff, mask):
    global _PROG
    if _PROG is None:
        _PROG = _build_program()

    x = np.asarray(x, dtype=np.float32).reshape(ROWS, K)
    base = np.asarray(base, dtype=np.float32)
    mask = np.asarray(mask, dtype=np.int32)

    in_maps, shard_ids = make_in_maps(x, base, coeff, mask)
    res = run_bass_kernel_spmd(_PROG, in_maps, list(range(8))).results

    out = np.empty((ROWS, D_OUT), dtype=np.float32)
    for i, (r, c) in enumerate(shard_ids):
        out[r * M:(r + 1) * M, c * NC:(c + 1) * NC] = \
            np.asarray(res[i]["out"]).astype(np.float32)
    return out.reshape(B, S, D_OUT)



# revision 22
# speedup vs baseline: 1.0882x; 1.0526x over previous
"""BinaryDiff kernel for Trainium2 (8 NeuronCores) — bf16 end-to-end.

Computes out = x @ base + coeff * (x @ (2*mask - 1)) by folding the two
matmuls into one:  out = x @ W,  W = base + coeff*(2*mask - 1).

Sharding (8 cores = 2 row-groups x 4 col-groups):
  - x rows (B*S = 8192) split in 2 -> each core gets an x^T shard
    [4096 K, 4096 rows], pre-arranged on host in slab-major layout AND
    pre-cast to bf16, so every slab DMA is contiguous per partition and
    PE consumes it directly (no on-device conversion).
  - base/mask cols (4096) split in 4 -> per-core shards [4096, 1024]
    (base bf16, mask int8)
  - each core computes out shard [4096, 1024] fp32; host concatenates.

On-device per core:
  - W = bf16(bf16(base) + (2c*mask - c)) built once into resident SBUF
    ([128,32,1024] bf16) via ACT affine (int8->f32, runtime coeff via
    scale/bias APs) + DVE add.
  - x^T bf16 slabs [128,32,128] DMA'd straight into matmul position;
    32 m-strips x 2 n-halves x 32 k-chunks of bf16 matmuls (moving dim
    512) accumulate in fp32 across 8 PSUM banks.
  - ACT copies PSUM->SBUF (fp32), gpsimd DMAs results out.

Raw bass with manual semaphores. Two hard rules learned on this stack:
  1. Engine datapath instructions may carry at most ONE sync wait, so
     every wait is a standalone wait_ge on the consuming engine.
  2. DMA completions across different HW queues are unordered, so a
     cumulative semaphore over many in-flight DMAs is racy. DMAs use
     per-lane semaphores with at most one outstanding DMA per lane
     (enforced by the consumer-side slot gating). Engine completions
     retire in order, so cumulative per-engine semaphores are sound.

Startup-path optimizations (timeline-sim showed a single ~30us PE gap
at rep start, everything after it gap-free at the PE roofline):
  - DMA issue order interleaves the first W pieces with the 4 chase
    slabs (PE's first matmul needs strips 0-3 AND piece 0; queueing all
    8 slabs ahead of the first base/mask DMA stalled the PE).
  - 12 dummy matmuls on garbage SBUF warm the PE pstate during the
    startup DMA wait (ps[7] is reset by group 7's start=True, so the
    garbage is never observable).

Note: fp8e4 DoubleRow (2 k-tiles/instr @ 0.5 cyc/row nominal) was fully
prototyped with 3-term error compensation (xhi@Whi + xlo@Whi + xhi@Wlo,
rel err 2.9e-3) but measured SLOWER than bf16 on this hardware: a DR
matmul's 256-row stationary load serializes with its moving stream
(~200ns/instr vs bf16's 214 at 512 moving rows), so the 1.5x instruction
count of the compensated scheme loses; uncompensated fp8 fails the 2e-2
gate (~1e-1 full-scale). See kernel_fp8c.py.
"""
import contextlib

import numpy as np
import ml_dtypes

import concourse.bass as bass
import concourse.mybir as mybir
from concourse.bass_utils import run_bass_kernel_spmd

f32 = mybir.dt.float32
bf16 = mybir.dt.bfloat16
i8 = mybir.dt.int8
Copy = mybir.ActivationFunctionType.Copy
Identity = mybir.ActivationFunctionType.Identity

P = 128
B, S, D_IN, D_OUT = 4, 2048, 4096, 4096
ROWS = B * S                  # 8192
R_SHARDS, C_SHARDS = 2, 4
M = ROWS // R_SHARDS          # 4096 rows per core
NC = D_OUT // C_SHARDS        # 1024 cols per core
K = D_IN                      # 4096 contraction
KT = K // P                   # 32 k-chunks
MS = M // P                   # 32 m-strips
NH = NC // 512                # 2 n-halves
NT = 512
N_PIECES = KT                 # 32 W build pieces (one full-width [128,1024] per k)
N_GROUPS = MS * NH            # 64 output groups
SLAB_BUFS = 8
KH = KT // 2                  # k-chunks per half-slab DMA
CHUNK_BUFS = 4
OUT_BUFS = 4
PSB = 8                       # psum banks in rotation
XT_LANES = 16                 # half-slab DMA sem lanes (2 per slab buffer;
                              # 16 outstanding 512KB DMAs keep more rings busy)
PIECE_LANES = 8               # W piece DMA sem lanes (> CHUNK_BUFS)
OD_LANES = 8                  # out DMA sem lanes (> OUT_BUFS)


def _build_program(reps=1, kmult=1, out_f32=False):
    """reps > 1 repeats the whole pipeline inside one NEFF (for timing:
    T(reps=a) - T(reps=b) isolates (a-b) kernel bodies from dispatch
    overhead). Functionally identical output (each rep overwrites out).
    kmult > 1 issues every matmul kmult times (PE-rate probe; output is
    kmult times too large — timing use only)."""
    out_dt = f32 if out_f32 else bf16
    nc = bass.Bass()
    # xT arrives in slab-major bf16: xT_host[s, p, ko, i] = x[s*128+i, ko*128+p]
    # so each slab DMA reads 128 partitions x 8KB fully contiguous.
    xT = nc.declare_dram_parameter("xT", [MS * P, KT * P], bf16, isOutput=False)
    base = nc.declare_dram_parameter("base", [K, NC], bf16, isOutput=False)
    mask = nc.declare_dram_parameter("mask", [K, NC], i8, isOutput=False)
    coeff = nc.declare_dram_parameter("coeff", [P, 1], f32, isOutput=False)
    out = nc.declare_dram_parameter("out", [M, NC], out_dt, isOutput=True)

    xT3 = xT.rearrange("(s p) (ko i) -> s p ko i", p=P, i=P)
    base3 = base.rearrange("(ko p) n -> p ko n", p=P)
    mask3 = mask.rearrange("(ko p) n -> p ko n", p=P)
    out3 = out.rearrange("(mo p) n -> p mo n", p=P)

    with contextlib.ExitStack() as ctx:
        s_cdma = ctx.enter_context(nc.semaphore("s_cdma"))
        s_c2 = ctx.enter_context(nc.semaphore("s_c2"))
        s_xt = [ctx.enter_context(nc.semaphore(f"s_xt{i}"))
                for i in range(XT_LANES)]
        s_b = [ctx.enter_context(nc.semaphore(f"s_b{i}"))
               for i in range(PIECE_LANES)]
        s_m = [ctx.enter_context(nc.semaphore(f"s_m{i}"))
               for i in range(PIECE_LANES)]
        s_od = [ctx.enter_context(nc.semaphore(f"s_od{i}"))
                for i in range(OD_LANES)]
        s_s = ctx.enter_context(nc.semaphore("s_s"))      # ACT s-op done (1/piece)
        s_w = ctx.enter_context(nc.semaphore("s_w"))      # DVE w-op done (1/piece)
        s_mm = ctx.enter_context(nc.semaphore("s_mm"))    # PE group done (1/group)
        s_oc = ctx.enter_context(nc.semaphore("s_oc"))    # ACT out-copy done (1/group)

        w_sb = ctx.enter_context(nc.sbuf_tensor("w_sb", [P, KT, NC], bf16))
        xt_sb = ctx.enter_context(
            nc.sbuf_tensor("xt_sb", [P, SLAB_BUFS, KT, P], bf16))
        b_sb = ctx.enter_context(nc.sbuf_tensor("b_sb", [P, CHUNK_BUFS, NC], bf16))
        m_sb = ctx.enter_context(nc.sbuf_tensor("m_sb", [P, CHUNK_BUFS, NC], i8))
        sa_sb = ctx.enter_context(
            nc.sbuf_tensor("sa_sb", [P, CHUNK_BUFS, NC], bf16))
        o_sb = ctx.enter_context(
            nc.sbuf_tensor("o_sb", [P, OUT_BUFS, NT], out_dt))
        c_sb = ctx.enter_context(nc.sbuf_tensor("c_sb", [P, 1], f32))
        c2_sb = ctx.enter_context(nc.sbuf_tensor("c2_sb", [P, 1], f32))
        cn_sb = ctx.enter_context(nc.sbuf_tensor("cn_sb", [P, 1], f32))
        ps = [
            ctx.enter_context(nc.psum_tensor(f"ps{i}", [P, NT], f32))
            for i in range(PSB)
        ]

        with nc.Block() as block:

            @block.sync
            def _(sync):
                sync.dma_start(c_sb[:], coeff[:]).then_inc(s_cdma, 16)
                for it in range(reps):
                    bW = it * N_PIECES          # s_s/s_w base
                    bX = it * MS                # slab count base
                    bG = it * N_GROUPS
                    if it > 0:
                        # serialize rep boundaries so per-body timing equals a
                        # single-shot run (also keeps w_sb write/read ordered)
                        sync.wait_ge(s_oc, it * N_GROUPS)
                    # first slabs of this rep; slot s%SLAB_BUFS previously
                    # held strip s-SLAB_BUFS, free once PE finished its
                    # NH groups (s_mm counts one per group, in order).
                    # Each slab is fetched as two half-slab DMAs on separate
                    # lanes so twice as many rings run concurrently.
                    # Startup order interleaves the first W pieces with the
                    # chase slabs: the PE's first matmul needs strips 0-3 AND
                    # piece 0, so queueing all 8 slabs (8MB) ahead of the
                    # first base/mask DMA stalls the PE ~30us at rep start.
                    def emit_slab(s):
                        if bX + s >= SLAB_BUFS:
                            sync.wait_ge(s_mm, NH * (bX + s - SLAB_BUFS + 1))
                        for hf in range(2):
                            sync.dma_start(
                                xt_sb[:, s % SLAB_BUFS,
                                      hf * KH:(hf + 1) * KH],
                                xT3[s][:, hf * KH:(hf + 1) * KH],
                            ).then_inc(s_xt[(2 * s + hf) % XT_LANES], 16)

                    def emit_piece(j):
                        if bW + j >= CHUNK_BUFS:
                            sync.wait_ge(s_w, bW + j - CHUNK_BUFS + 1)
                            sync.wait_ge(s_s, bW + j - CHUNK_BUFS + 1)
                        sync.dma_start(
                            b_sb[:, j % CHUNK_BUFS], base3[:, j],
                        ).then_inc(s_b[j % PIECE_LANES], 16)
                        sync.dma_start(
                            m_sb[:, j % CHUNK_BUFS], mask3[:, j],
                        ).then_inc(s_m[j % PIECE_LANES], 16)

                    # pieces 0-1 and the 4 chase slabs first (PE's first
                    # need), then the full piece stream (feeds the chase at
                    # ~1.2us cadence), then slabs 4-7 (needed only after the
                    # chase) and the gated tail.
                    for j in range(2):
                        emit_piece(j)
                    for s in range(4):
                        emit_slab(s)
                    for j in range(2, N_PIECES):
                        emit_piece(j)
                    for s in range(4, min(SLAB_BUFS, MS)):
                        emit_slab(s)
                    # remaining slabs (two half-DMAs each)
                    for s in range(SLAB_BUFS, MS):
                        emit_slab(s)

            @block.scalar
            def _(scalar):
                scalar.wait_ge(s_cdma, 16)
                scalar.activation(c2_sb[:], c_sb[:], Copy, scale=2.0)
                scalar.activation(cn_sb[:], c_sb[:], Copy, scale=-1.0) \
                    .then_inc(s_c2, 1)
                # scale/bias operands are fetched at dispatch; wait for our own
                # writes to drain before the first use
                scalar.wait_ge(s_c2, 1)
                for it in range(reps):
                    bW = it * N_PIECES
                    bG = it * N_GROUPS
                    bP = it * (N_PIECES // PIECE_LANES) * 16
                    for j in range(N_PIECES):
                        scalar.wait_ge(s_m[j % PIECE_LANES],
                                       bP + 16 * (j // PIECE_LANES + 1))
                        if bW + j >= CHUNK_BUFS:
                            scalar.wait_ge(s_w, bW + j - CHUNK_BUFS + 1)
                        scalar.activation(
                            sa_sb[:, j % CHUNK_BUFS], m_sb[:, j % CHUNK_BUFS],
                            Identity, scale=c2_sb[:], bias=cn_sb[:],
                        ).then_inc(s_s, 1)
                    # PSUM -> SBUF copies
                    for g in range(N_GROUPS):
                        scalar.wait_ge(s_mm, bG + g + 1)
                        if bG + g >= OUT_BUFS:
                            gp = bG + g - OUT_BUFS
                            scalar.wait_ge(s_od[gp % OD_LANES],
                                           16 * (gp // OD_LANES + 1))
                        scalar.copy(o_sb[:, g % OUT_BUFS], ps[g % PSB][:]) \
                            .then_inc(s_oc, 1)

            @block.vector
            def _(vector):
                for it in range(reps):
                    bW = it * N_PIECES
                    bP = it * (N_PIECES // PIECE_LANES) * 16
                    for j in range(N_PIECES):
                        vector.wait_ge(s_s, bW + j + 1)
                        vector.wait_ge(s_b[j % PIECE_LANES],
                                       bP + 16 * (j // PIECE_LANES + 1))
                        vector.tensor_tensor(
                            w_sb[:, j, :],
                            sa_sb[:, j % CHUNK_BUFS], b_sb[:, j % CHUNK_BUFS],
                            mybir.AluOpType.add,
                        ).then_inc(s_w, 1)

            @block.tensor
            def _(tensor):
                # pstate warmup: dummy matmuls on garbage SBUF during the
                # startup DMA gap so the first real matmuls run at max clock.
                # ps[7]'s first real use is group 7 with start=True, which
                # resets the bank, so the garbage results are never read.
                for d in range(12):
                    tensor.matmul(
                        ps[7][:], xt_sb[:, 7, d, :], w_sb[:, d, :NT],
                        start=True, stop=True, skip_group_check=True,
                    )
                for it in range(reps):
                    bW = it * N_PIECES
                    bX = it * MS
                    bG = it * N_GROUPS
                    bL = it * (2 * MS // XT_LANES) * 16
                    # strips 0-3 fused k-major across all 8 psum banks: 8 mms
                    # of PE work per W piece keeps PE busy while the
                    # W build streams in
                    NCH = PSB // NH   # chase strips
                    for st in range(NCH):
                        tensor.wait_ge(s_xt[(2 * st) % XT_LANES], bL + 16)
                        tensor.wait_ge(s_xt[(2 * st + 1) % XT_LANES],
                                       bL + 16)
                    for k in range(KT):
                        tensor.wait_ge(s_w, bW + k + 1)
                        for st in range(NCH):
                            for h in range(NH):
                                g = bG + NH * st + h
                                if k == 0 and g >= PSB:
                                    tensor.wait_ge(s_oc, g - PSB + 1)
                                for q in range(kmult):
                                    mm = tensor.matmul(
                                        ps[g % PSB][:], xt_sb[:, st, k, :],
                                        w_sb[:, k, h * NT:(h + 1) * NT],
                                        start=(k == 0 and q == 0),
                                        stop=(k == KT - 1 and q == kmult - 1),
                                    )
                                    if k == KT - 1 and q == kmult - 1:
                                        # stops fire in group order 0..7
                                        mm.then_inc(s_mm, 1)
                    for strip in range(NCH, MS):
                        tensor.wait_ge(s_xt[(2 * strip) % XT_LANES],
                                       bL + 16 * (strip // SLAB_BUFS + 1))
                        tensor.wait_ge(s_xt[(2 * strip + 1) % XT_LANES],
                                       bL + 16 * (strip // SLAB_BUFS + 1))
                        for h in range(NH):
                            g = bG + NH * strip + h
                            if g >= PSB:
                                tensor.wait_ge(s_oc, g - PSB + 1)
                            for k in range(KT):
                                for q in range(kmult):
                                    mm = tensor.matmul(
                                        ps[g % PSB][:],
                                        xt_sb[:, strip % SLAB_BUFS, k, :],
                                        w_sb[:, k, h * NT:(h + 1) * NT],
                                        start=(k == 0 and q == 0),
                                        stop=(k == KT - 1 and q == kmult - 1),
                                    )
                                    if k == KT - 1 and q == kmult - 1:
                                        mm.then_inc(s_mm, 1)

            @block.gpsimd
            def _(gpsimd):
                for it in range(reps):
                    bG = it * N_GROUPS
                    for g in range(N_GROUPS):
                        strip, h = g // NH, g % NH
                        gpsimd.wait_ge(s_oc, bG + g + 1)
                        gpsimd.dma_start(
                            out3[:, strip, h * NT:(h + 1) * NT],
                            o_sb[:, g % OUT_BUFS],
                        ).then_inc(s_od[g % OD_LANES], 16)
                for i in range(OD_LANES):
                    cnt = (reps * N_GROUPS - 1 - i) // OD_LANES + 1
                    gpsimd.wait_ge(s_od[i], 16 * cnt)

    return nc


def make_in_maps(x, base, coeff, mask):
    """Host-side shard + layout prep. x: [ROWS, K] f32 (already reshaped)."""
    in_maps = []
    shard_ids = []
    for r in range(R_SHARDS):
        x_r = x[r * M:(r + 1) * M, :]
        # slab-major: [s, p, ko, i] = x_r[s*128+i, ko*128+p], cast to bf16
        xT_r = np.ascontiguousarray(
            x_r.reshape(MS, P, KT, P).transpose(0, 3, 2, 1)
        ).reshape(MS * P, KT * P).astype(ml_dtypes.bfloat16)
        for c in range(C_SHARDS):
            in_maps.append({
                "xT": xT_r,
                "base": np.ascontiguousarray(
                    base[:, c * NC:(c + 1) * NC]).astype(ml_dtypes.bfloat16),
                "mask": np.ascontiguousarray(
                    mask[:, c * NC:(c + 1) * NC]).astype(np.int8),
                "coeff": np.full((P, 1), np.float32(coeff), dtype=np.float32),
            })
            shard_ids.append((r, c))
    return in_maps, shard_ids


_PROG = None


def kernel(x, base, coeff, mask):
    global _PROG
    if _PROG is None:
        _PROG = _build_program()

    x = np.asarray(x, dtype=np.float32).reshape(ROWS, K)
    base = np.asarray(base, dtype=np.float32)
    mask = np.asarray(mask, dtype=np.int32)

    in_maps, shard_ids = make_in_maps(x, base, coeff, mask)
    res = run_bass_kernel_spmd(_PROG, in_maps, list(range(8))).results

    out = np.empty((ROWS, D_OUT), dtype=np.float32)
    for i, (r, c) in enumerate(shard_ids):
        out[r * M:(r + 1) * M, c * NC:(c + 1) * NC] = \
            np.asarray(res[i]["out"]).astype(np.float32)
    return out.reshape(B, S, D_OUT)

